# revision 24
# baseline (speedup 1.0000x reference)
"""DeepSeek sparse attention TRN2 kernel: 8-core query-parallel.

Hardcoded for B=1, S=768, E=512, H=8, DK=64, TOPK=384, 8 cores.
  - Core c owns queries [96c, 96c+96). Output = host concat of per-core rows.
  - Indexer chain in fp32 matmuls (top-k set needs ~1e-5 score accuracy).
  - Top-k via per-row threshold: 16 bisection steps with ACT Sign+accum
    counting, then exact top-16 fixup (max8 + match_replace + max8).
    Tie-break ramp -t*2^-40 reproduces lax.top_k's lower-index-first
    ordering on the exact-zero relu atom.
  - Attention = dense QK^T + multiplicative 0/1 mask (math-identical to
    gather+softmax over the selected set), bf16.
  - bk dropped (softmax shift-invariance); bv folded into bo2 on host.
"""
import numpy as np
import ml_dtypes

S, E, H, DK = 768, 512, 8, 64
NQ = 96
NC = 8
KCH = 4            # 512/128
TCH = 6            # 768/128
TH = 384           # t-half for fp32 PSUM-bank-sized N
SCALING = 1.0 / np.sqrt(DK)
RAMP_EPS = float(2.0 ** -40)
R_ITERS = 12
BRK = 16.0
NEG = -1e30


def build_nc(stage=99):
    import concourse.bass as bass
    import concourse.bacc as bacc
    from concourse import mybir
    from concourse.tile import TileContext

    f32 = mybir.dt.float32
    bf16 = mybir.dt.bfloat16
    AF = mybir.ActivationFunctionType
    OP = mybir.AluOpType

    nc = bacc.Bacc("TRN2", target_bir_lowering=False, debug=False)

    def din(name, shape, dt):
        return nc.dram_tensor(name, shape, dt, kind="ExternalInput")

    xT = din("xT", [E, S], f32)
    xT16 = din("xT16", [E, S], bf16)
    iqW = din("iqW", [E, E], f32)
    ikW = din("ikW", [E, DK], f32)
    wpW = din("wpW", [E, H], f32)
    wq16 = din("wq16", [E, E], bf16)
    wk16 = din("wk16", [E, E], bf16)
    wv16 = din("wv16", [E, E], bf16)
    wo16 = din("wo16", [DK, H, E], bf16)
    iqb = din("iqb", [E, 1], f32)
    ikb = din("ikb", [DK, 1], f32)
    wpb = din("wpb", [H, 1], f32)
    bqh = din("bqh", [DK, H], f32)
    bo2 = din("bo2", [1, E], f32)
    bd01 = din("bd01", [128, 160], f32)
    ramp = din("ramp", [1, S], f32)
    col16 = din("col16", [1, 16], f32)
    ident = din("ident", [128, 128], f32)
    xTq = din("xTq", [E, NQ], f32)
    xTq16 = din("xTq16", [E, NQ], bf16)
    out = nc.dram_tensor("out", [NQ, E], f32, kind="ExternalOutput")
    dbg = nc.dram_tensor("dbg", [NQ, S], f32, kind="ExternalOutput")
    wT_dram = nc.dram_tensor("wT_dram", [H, NQ], f32)
    den_dram = nc.dram_tensor("den_dram", [2, 4 * NQ], f32)

    def bcastP(ap, p):
        return bass.AP(tensor=ap.tensor, offset=ap.offset,
                       ap=[[0, p]] + ap.ap[1:])

    import contextlib
    with TileContext(nc) as tc:
      with contextlib.suppress(StopIteration):
        with tc.tile_pool(name="w1", bufs=1) as w1, \
             tc.tile_pool(name="big", bufs=1) as big, \
             tc.tile_pool(name="scp", bufs=2) as scp, \
             tc.tile_pool(name="tiny", bufs=1) as tiny, \
             tc.tile_pool(name="psA", bufs=3, space="PSUM") as psA, \
             tc.tile_pool(name="psB", bufs=1, space="PSUM") as psB:

            # ---------------- loads (chunked [128, k, n]) ----------------
            s_xT = w1.tile([128, KCH, S], f32)
            s_xT16 = w1.tile([128, KCH, S], bf16)
            s_xTq = w1.tile([128, KCH, NQ], f32)
            s_xTq16 = w1.tile([128, KCH, NQ], bf16)
            s_iqW = w1.tile([128, KCH, E], f32)
            s_ikW = w1.tile([128, KCH, DK], f32)
            s_wpW = w1.tile([128, KCH, H], f32)
            s_wq = w1.tile([128, KCH, E], bf16)
            s_wk = w1.tile([128, KCH, E], bf16)
            s_wv = w1.tile([128, KCH, E], bf16)
            s_wo = w1.tile([DK, H, E], bf16)
            s_iqb = w1.tile([128, KCH], f32)
            s_bqh = w1.tile([DK, H], f32)
            s_ikb = w1.tile([DK, 1], f32)
            s_wpb = w1.tile([H, 1], f32)
            s_bd01 = w1.tile([128, 160], f32)
            s_ident = w1.tile([128, 128], f32)
            s_ramp = w1.tile([NQ, S], f32)
            s_col16 = w1.tile([NQ, 16], f32)
            s_bo2 = w1.tile([NQ, E], f32)

            for dst, src in [(s_xT, xT), (s_xT16, xT16), (s_xTq, xTq),
                             (s_xTq16, xTq16), (s_iqW, iqW), (s_ikW, ikW),
                             (s_wpW, wpW), (s_wq, wq16), (s_wk, wk16),
                             (s_wv, wv16)]:
                nc.sync.dma_start(
                    out=dst, in_=src[:, :].rearrange("(k p) n -> p k n", p=128))
            nc.sync.dma_start(
                out=s_iqb, in_=iqb[:, :].rearrange("(k p) o -> p (k o)", p=128))
            nc.sync.dma_start(out=s_wo, in_=wo16[:, :, :])
            nc.sync.dma_start(out=s_bqh, in_=bqh[:, :])
            nc.sync.dma_start(out=s_ikb, in_=ikb[:, :])
            nc.sync.dma_start(out=s_wpb, in_=wpb[:, :])
            nc.sync.dma_start(out=s_bd01, in_=bd01[:, :])
            nc.sync.dma_start(out=s_ident, in_=ident[:, :])
            nc.sync.dma_start(out=s_ramp, in_=bcastP(ramp[:, :], NQ))
            nc.sync.dma_start(out=s_col16, in_=bcastP(col16[:, :], NQ))
            nc.sync.dma_start(out=s_bo2, in_=bcastP(bo2[:, :], NQ))

            # =========== INDEXER (fp32) ===========
            s_kidT = big.tile([DK, S], f32)
            for th in range(2):
                pk = psA.tile([DK, TH], f32, tag="ps")
                for k in range(KCH):
                    nc.tensor.matmul(pk, s_ikW[:, k, :],
                                     s_xT[:, k, TH * th:TH * (th + 1)],
                                     start=(k == 0), stop=(k == KCH - 1))
                nc.scalar.activation(out=s_kidT[:, TH * th:TH * (th + 1)],
                                     in_=pk, func=AF.Identity, bias=s_ikb)

            s_qidT = big.tile([128, KCH, NQ], f32)
            for m in range(KCH):
                pq = psA.tile([128, NQ], f32, tag="ps")
                for k in range(KCH):
                    nc.tensor.matmul(pq, s_iqW[:, k, 128 * m:128 * (m + 1)],
                                     s_xTq[:, k, :],
                                     start=(k == 0), stop=(k == KCH - 1))
                nc.scalar.activation(out=s_qidT[:, m, :], in_=pq,
                                     func=AF.Identity,
                                     bias=s_iqb[:, m:m + 1])

            s_widT = tiny.tile([H, NQ], f32)
            pw = psA.tile([H, NQ], f32, tag="ps")
            for k in range(KCH):
                nc.tensor.matmul(pw, s_wpW[:, k, :], s_xTq[:, k, :],
                                 start=(k == 0), stop=(k == KCH - 1))
            nc.scalar.activation(out=s_widT, in_=pw, func=AF.Identity,
                                 bias=s_wpb)
            nc.sync.dma_start(out=wT_dram[:, :], in_=s_widT)

            # score lhsT tiles [64, 128]: col = 32*hl + s  (hl-major)
            # rows d; head h = 4*hf + hl; queries s in group g (32 wide)
            sc_lhs = [[tiny.tile([DK, 128], f32, tag=f"sclhs_{g}_{hf}", name=f"sclhs_{g}_{hf}")
                       for hf in range(2)] for g in range(3)]
            for g in range(3):
                for hf in range(2):
                    for hl in range(4):
                        h = 4 * hf + hl
                        m, r = h // 2, (h % 2) * DK
                        nc.sync.dma_start(
                            out=sc_lhs[g][hf][:, 32 * hl:32 * (hl + 1)],
                            in_=s_qidT[r:r + DK, m, 32 * g:32 * (g + 1)])

            # w columns [128,1]: partition 32*hl+s -> w[32g+s, 4hf+hl]
            w_cols = [[tiny.tile([128, 1], f32, tag=f"wcol_{g}_{hf}", name=f"wcol_{g}_{hf}")
                       for hf in range(2)] for g in range(3)]
            for g in range(3):
                for hf in range(2):
                    for hl in range(4):
                        nc.sync.dma_start(
                            out=w_cols[g][hf][32 * hl:32 * (hl + 1), :],
                            in_=wT_dram[4 * hf + hl:4 * hf + hl + 1,
                                        32 * g:32 * (g + 1)])

            # scores + relu*w
            ws = [[[scp.tile([128, TH], f32, tag=f"ws_{g}_{hf}_{th}", name=f"ws_{g}_{hf}_{th}")
                    for th in range(2)] for hf in range(2)] for g in range(3)]
            for g in range(3):
                for hf in range(2):
                    for th in range(2):
                        psc = psA.tile([128, TH], f32, tag="ps")
                        nc.tensor.matmul(psc, sc_lhs[g][hf],
                                         s_kidT[:, TH * th:TH * (th + 1)],
                                         start=True, stop=True)
                        nc.vector.scalar_tensor_tensor(
                            out=ws[g][hf][th], in0=psc, scalar=0.0,
                            in1=w_cols[g][hf].to_broadcast([128, TH]),
                            op0=OP.max, op1=OP.mult)

            # combine -> ind (with tie-break ramp subtracted)
            s_ind = big.tile([NQ, S], f32)
            for th in range(2):
                pind = psB.tile([NQ, TH], f32, tag="pind")
                first = True
                for g in range(3):
                    for hf in range(2):
                        nc.tensor.matmul(
                            pind, s_bd01[:, 64 - 32 * g:160 - 32 * g],
                            ws[g][hf][th], start=first,
                            stop=(g == 2 and hf == 1))
                        first = False
                nc.vector.tensor_sub(s_ind[:, TH * th:TH * (th + 1)], pind,
                                     s_ramp[:, TH * th:TH * (th + 1)])

            if stage < 90:
                nc.sync.dma_start(out=dbg[:, :], in_=s_ind)
            if stage < 2:
                s_o0 = big.tile([NQ, E], f32, name="s_o0")
                nc.vector.memset(s_o0, 0.0)
                nc.sync.dma_start(out=out[:, :], in_=s_o0)
                raise StopIteration
            # =========== TOPK threshold ===========
            lo = tiny.tile([NQ, 1], f32)
            hi = tiny.tile([NQ, 1], f32)
            tmp = tiny.tile([NQ, 1], f32)
            nmid = tiny.tile([NQ, 1], f32)
            mid = tiny.tile([NQ, 1], f32)
            u8 = mybir.dt.uint8
            cmp = tiny.tile([NQ, 1], u8)
            ncmp = tiny.tile([NQ, 1], u8)
            acc = tiny.tile([NQ, 1], f32)
            sgn_scr = big.tile([NQ, S], f32)
            nc.vector.memset(lo, -BRK)
            nc.vector.memset(hi, BRK)
            for r in range(R_ITERS):
                nc.vector.tensor_add(tmp, lo, hi)
                nc.vector.tensor_scalar_mul(nmid, tmp, -0.5)
                nc.scalar.activation(out=sgn_scr, in_=s_ind, func=AF.Sign,
                                     bias=nmid, scale=1.0, accum_out=acc)
                nc.vector.tensor_scalar(cmp, acc, 0.0, None, op0=OP.is_ge)
                nc.vector.tensor_scalar(ncmp, acc, 0.0, None, op0=OP.is_lt)
                nc.vector.tensor_scalar_mul(mid, nmid, -1.0)
                nc.vector.copy_predicated(lo, cmp, mid)
                nc.vector.copy_predicated(hi, ncmp, mid)

            # exact count at hi; in-bracket top-16
            scr_b = big.tile([NQ, S], bf16, tag="scr_b")
            c_hi = tiny.tile([NQ, 1], f32)
            nc.vector.tensor_scalar(scr_b, s_ind, hi, None, op0=OP.is_ge,
                                    op1=OP.add, accum_out=c_hi)
            inb_lo = big.tile([NQ, S], f32, tag="inb_lo")
            inb_hi = big.tile([NQ, S], f32, tag="inb_hi")
            nc.vector.tensor_scalar(inb_lo, s_ind, lo, None, op0=OP.is_ge)
            nc.vector.tensor_scalar(inb_hi, s_ind, hi, None, op0=OP.is_lt)
            inb = big.tile([NQ, S], mybir.dt.uint8, tag="inb")
            nc.vector.tensor_mul(inb, inb_lo, inb_hi)
            mlo = big.tile([NQ, S], f32, tag="mlo")
            nc.vector.memset(mlo, NEG)
            nc.vector.copy_predicated(mlo, inb, s_ind)
            m16 = tiny.tile([NQ, 16], f32)
            mlo2 = big.tile([NQ, S], f32, tag="mlo2")
            nc.vector.max(out=m16[:, 0:8], in_=mlo)
            nc.vector.match_replace(out=mlo2, in_to_replace=m16[:, 0:8],
                                    in_values=mlo, imm_value=NEG)
            nc.vector.max(out=m16[:, 8:16], in_=mlo2)
            need_m1 = tiny.tile([NQ, 1], f32)
            nc.vector.tensor_scalar(need_m1, c_hi, -1.0, 383.0, op0=OP.mult,
                                    op1=OP.add)
            oh = tiny.tile([NQ, 16], f32)
            oh2 = tiny.tile([NQ, 16], f32)
            tstar = tiny.tile([NQ, 1], f32)
            nc.vector.tensor_scalar(oh, s_col16, need_m1, None, op0=OP.is_equal)
            nc.vector.scalar_tensor_tensor(out=oh2, in0=m16, scalar=1.0,
                                           in1=oh, op0=OP.mult, op1=OP.mult,
                                           accum_out=tstar)
            mask01 = big.tile([NQ, S], bf16, tag="mask01")
            nc.vector.tensor_scalar(mask01, s_ind, tstar, None, op0=OP.is_ge)
            # transpose mask -> maskT [128, 6, 96]
            s_maskT = big.tile([128, TCH, NQ], bf16)
            for t in range(TCH):
                nc.sync.dma_start_transpose(
                    s_maskT[:, t, :], mask01[:, 128 * t:128 * (t + 1)])

            if stage < 3:
                s_o1 = big.tile([NQ, E], f32, name="s_o1")
                nc.vector.memset(s_o1, 0.0)
                nc.vector.tensor_copy(s_o1[:, 0:1], tstar)
                nc.sync.dma_start(out=out[:, :], in_=s_o1)
                raise StopIteration
            # =========== ATTENTION (bf16) ===========
            s_KT = big.tile([DK, H, S], bf16)
            s_QT = big.tile([DK, H, NQ], bf16)
            for h in range(H):
                for th in range(2):
                    pk2 = psA.tile([DK, TH], f32, tag="ps")
                    for k in range(KCH):
                        nc.tensor.matmul(pk2,
                                         s_wk[:, k, DK * h:DK * (h + 1)],
                                         s_xT16[:, k, TH * th:TH * (th + 1)],
                                         start=(k == 0), stop=(k == KCH - 1))
                    nc.scalar.copy(s_KT[:, h, TH * th:TH * (th + 1)], pk2)
                pq2 = psA.tile([DK, NQ], f32, tag="ps")
                for k in range(KCH):
                    nc.tensor.matmul(pq2, s_wq[:, k, DK * h:DK * (h + 1)],
                                     s_xTq16[:, k, :],
                                     start=(k == 0), stop=(k == KCH - 1))
                nc.scalar.activation(out=s_QT[:, h, :], in_=pq2,
                                     func=AF.Identity, bias=s_bqh[:, h:h + 1])
            s_V = big.tile([128, TCH, E], bf16)
            for t in range(TCH):
                pv = psA.tile([128, E], f32, tag="ps")
                for k in range(KCH):
                    nc.tensor.matmul(pv, s_xT16[:, k, 128 * t:128 * (t + 1)],
                                     s_wv[:, k, :],
                                     start=(k == 0), stop=(k == KCH - 1))
                nc.scalar.copy(s_V[:, t, :], pv)

            if stage < 4:
                s_o2 = big.tile([NQ, E], f32, name="s_o2")
                nc.vector.memset(s_o2, 0.0)
                nc.vector.tensor_copy(s_o2[:64, :96],
                                      s_V[:64, 0, :96])
                nc.sync.dma_start(out=out[:, :], in_=s_o2)
                raise StopIteration
            # masked softmax numerators + denominators
            w_tiles = [[scp.tile([128, 4 * NQ], bf16, tag=f"wt_{t}_{q}", name=f"wt_{t}_{q}") for q in range(2)] for t in range(TCH)]
            pden = [psB.tile([1, 4 * NQ], f32, tag=f"pden{q}", name=f"pden{q}")
                    for q in range(2)]
            onesrow = tiny.tile([128, 1], bf16)
            nc.vector.memset(onesrow, 1.0)
            for t in range(TCH):
                for q in range(2):
                    psc2 = psA.tile([128, 4 * NQ], f32, tag="ps")
                    for hl in range(4):
                        h = 4 * q + hl
                        nc.tensor.matmul(
                            psc2[:, NQ * hl:NQ * (hl + 1)],
                            s_KT[:, h, 128 * t:128 * (t + 1)],
                            s_QT[:, h, :],
                            start=True, stop=True)
                    wt = w_tiles[t][q]
                    nc.scalar.activation(out=wt, in_=psc2, func=AF.Exp,
                                         scale=SCALING)
                    msl = s_maskT[:, t, :]
                    for hl in range(4):
                        nc.vector.tensor_mul(
                            wt[:, NQ * hl:NQ * (hl + 1)],
                            wt[:, NQ * hl:NQ * (hl + 1)], msl)
                    if stage == 47 and t == 0 and q == 0:
                        s_o5 = big.tile([NQ, E], f32, name="s_o5")
                        nc.vector.memset(s_o5, 0.0)
                        nc.vector.tensor_copy(s_o5[:, :384], wt[:96, :])
                        nc.sync.dma_start(out=out[:, :], in_=s_o5)
                        raise StopIteration
            if stage < 45:
                s_o4 = big.tile([NQ, E], f32, name="s_o4")
                nc.vector.memset(s_o4, 0.0)
                nc.vector.tensor_copy(s_o4[:, :384],
                                      w_tiles[0][0][:96, :])
                nc.sync.dma_start(out=out[:, :], in_=s_o4)
                raise StopIteration
            for q in range(2):
                for t in range(TCH):
                    nc.tensor.matmul(pden[q], onesrow, w_tiles[t][q],
                                     start=(t == 0), stop=(t == TCH - 1))

            if stage == 48:
                s_o3 = big.tile([NQ, E], f32, name="s_o3")
                nc.vector.memset(s_o3, 0.0)
                nc.vector.tensor_copy(s_o3[:1, :384], pden[0])
                nc.sync.dma_start(out=out[:, :], in_=s_o3)
                raise StopIteration
            s_den = tiny.tile([1, 4 * NQ], f32)
            s_den2 = tiny.tile([1, 4 * NQ], f32)
            nc.vector.reciprocal(s_den, pden[0])
            nc.vector.reciprocal(s_den2, pden[1])
            nc.sync.dma_start(out=den_dram[0:1, :], in_=s_den)
            nc.sync.dma_start(out=den_dram[1:2, :], in_=s_den2)

            s_attn = [big.tile([DK, NQ], bf16, tag=f"attn{h}", name=f"attn{h}")
                      for h in range(H)]
            for h in range(H):
                half = h % 2
                pa = psB.tile([DK, NQ], f32, tag=f"pa{half}")
                for t in range(TCH):
                    nc.tensor.matmul(
                        pa, s_V[:, t, DK * h:DK * (h + 1)],
                        w_tiles[t][h // 4][:, NQ * (h % 4):NQ * (h % 4 + 1)],
                        start=(t == 0), stop=(t == TCH - 1))
                rb = tiny.tile([DK, NQ], f32, tag=f"rb{half}")
                dsl = den_dram[h // 4:h // 4 + 1,
                               NQ * (h % 4):NQ * (h % 4 + 1)]
                nc.sync.dma_start(out=rb, in_=bcastP(dsl, DK))
                nc.vector.tensor_mul(s_attn[h], pa, rb)

            if stage == 49:
                s_o6 = big.tile([NQ, E], f32, name="s_o6")
                nc.vector.memset(s_o6, 0.0)
                nc.vector.tensor_copy(s_o6[:64, :96], s_attn[0])
                nc.vector.tensor_copy(s_o6[:64, 96:192], s_attn[7])
                nc.sync.dma_start(out=out[:, :], in_=s_o6)
                raise StopIteration
            po = psB.tile([NQ, E], f32, tag="pind")
            for h in range(H):
                nc.tensor.matmul(po, s_attn[h], s_wo[:, h, :],
                                 start=(h == 0), stop=(h == H - 1))
            s_out = big.tile([NQ, E], f32)
            nc.vector.tensor_add(s_out, po, s_bo2)
            nc.sync.dma_start(out=out[:, :], in_=s_out)

    nc.finalize()
    return nc


_NC_CACHE = {}


def _get_nc():
    if "nc" not in _NC_CACHE:
        _NC_CACHE["nc"] = build_nc()
    return _NC_CACHE["nc"]


def prep_inputs(x, Wq, bq_, Wk, bk_, Wv, bv_, Wo, bo_, iq_W, iq_b, ik_W, ik_b,
                wp_W, wp_b):
    bf = ml_dtypes.bfloat16
    f32 = np.float32
    xf = np.ascontiguousarray(np.asarray(x).reshape(S, E).astype(f32))
    xT = np.ascontiguousarray(xf.T)
    bd = np.zeros((128, 160), f32)
    for hl in range(4):
        for s_ in range(32):
            bd[32 * hl + s_, 64 + s_] = 1.0
    shared = {
        "xT": xT, "xT16": xT.astype(bf),
        "iqW": np.ascontiguousarray(iq_W, f32),
        "ikW": np.ascontiguousarray(ik_W, f32),
        "wpW": np.ascontiguousarray(wp_W, f32),
        "wq16": np.ascontiguousarray(Wq).astype(bf),
        "wk16": np.ascontiguousarray(Wk).astype(bf),
        "wv16": np.ascontiguousarray(Wv).astype(bf),
        "wo16": np.ascontiguousarray(
            np.asarray(Wo, f32).reshape(H, DK, E).transpose(1, 0, 2)).astype(bf),
        "iqb": np.ascontiguousarray(iq_b.reshape(E, 1), f32),
        "ikb": np.ascontiguousarray(ik_b.reshape(DK, 1), f32),
        "wpb": np.ascontiguousarray(wp_b.reshape(H, 1), f32),
        "bqh": np.ascontiguousarray(bq_.reshape(H, DK).T, f32),
        "bo2": np.ascontiguousarray(
            (np.asarray(bv_, np.float64) @ np.asarray(Wo, np.float64)
             + np.asarray(bo_, np.float64)).reshape(1, E)).astype(f32),
        "bd01": bd,
        "ramp": (np.arange(S, dtype=np.float64) * RAMP_EPS
                 ).astype(f32).reshape(1, S),
        "col16": np.arange(16, dtype=f32).reshape(1, 16),
        "ident": np.eye(128, dtype=np.float32),
    }
    in_maps = []
    for c in range(NC):
        m = dict(shared)
        xq = np.ascontiguousarray(xT[:, NQ * c:NQ * (c + 1)])
        m["xTq"] = xq
        m["xTq16"] = xq.astype(bf)
        in_maps.append(m)
    return in_maps


def kernel(**inputs):
    from concourse.bass_utils import run_bass_kernel_spmd
    nc = _get_nc()
    in_maps = prep_inputs(
        inputs["x"], inputs["Wq"], inputs["bq"], inputs["Wk"], inputs["bk"],
        inputs["Wv"], inputs["bv"], inputs["Wo"], inputs["bo"],
        inputs["iq_W"], inputs["iq_b"], inputs["ik_W"], inputs["ik_b"],
        inputs["wp_W"], inputs["wp_b"])
    res = run_bass_kernel_spmd(nc, in_maps, core_ids=list(range(NC)))
    outs = [res.results[c]["out"] for c in range(NC)]
    return np.concatenate(outs, axis=0)[None].astype(np.float32)
